# revision 8
# baseline (speedup 1.0000x reference)
"""Trainium2 Bass kernel for nn_LowPass: order-2 Butterworth filtfilt.

Strategy: the IIR's impulse response decays below fp32 noise within ~256
samples, so forward and backward passes are exact 256-tap FIR convolutions.
Each of the 8 cores owns 128 lanes (on SBUF partitions). Convolutions run on
the tensor engine as Toeplitz-structured matmuls in time-major layout:

  pass A: stream x, reduce per-lane max|x| (the clip bound; normalization
          commutes with the linear filter so no divide is needed:
          clip(y/s,-1,1)*s == clamp(y, -s, +s)).
  pass B: stream x -> PE transpose (time-major) -> MM1 (Toeplitz stationary,
          4 j-packed tiles, N=512) -> forward stream -> MM2 (forward tiles
          stationary, Toeplitz moving, N=256) -> clamp(+-s) -> out.

Odd-reflection padding (PADLEN=9) is assembled on-chip from the loaded edge
strips with negative-stride APs.
"""

import numpy as np

PADLEN = 9
T = 48000
LANES_TOTAL = 1024
N_CORES = 8
LANES = LANES_TOTAL // N_CORES  # 128 per core

KTAPS = 256
STRIP = 2048                # stream samples per strip
UNITS = STRIP // 128        # 16 tiles per strip
S_LEN = 49152               # padded stream length: 24 strips
NSTRIPS = S_LEN // STRIP    # 24
TP = T + 2 * PADLEN         # 48018 valid stream samples
NT_VALID = (TP + 127) // 128  # 376 tiles carry data (tile 375 partial: 18)
MM2_N = 256

# "f32r" (fast, ~1e-4 rel err) or "fp32" (exact, ~4x more PE time)
DT_MODE = "fp32"

_CACHE = {}


def _impulse_response(b, a, K):
    b = np.asarray(b, dtype=np.float64)
    a = np.asarray(a, dtype=np.float64)
    bn = b / a[0]
    an = a / a[0]
    h = np.zeros(K, dtype=np.float64)
    for t in range(K):
        acc = bn[t] if t < 3 else 0.0
        for i in range(1, 3):
            if t - i >= 0:
                acc -= an[i] * h[t - i]
        h[t] = acc
    return h


def _tables(b, a):
    h = _impulse_response(b, a, KTAPS)
    # MM1: fwd[t0j+m] = sum_k h[m + 256 - 128c - k] * S[t0j - 256 + 128c + k]
    toep1 = np.zeros((128, 3, 128), dtype=np.float32)  # [k][c][m]
    for c in range(3):
        for k in range(128):
            lo = max(0, 256 - 128 * c - k)
            for m in range(128):
                idx = m + 256 - 128 * c - k
                if 0 <= idx < KTAPS:
                    toep1[k, c, m] = h[idx]
    # MM2: bwd[t0+j2] = sum_k h[128c + k - j2] * fwd[t0 + 128c + k]
    toep2 = np.zeros((128, 4, MM2_N), dtype=np.float32)  # [k][c][j2]
    for c in range(4):
        for k in range(128):
            for j2 in range(MM2_N):
                idx = 128 * c + k - j2
                if 0 <= idx < KTAPS:
                    toep2[k, c, j2] = h[idx]
    return toep1.reshape(128, 3 * 128), toep2.reshape(128, 4 * MM2_N)


def _build(dt_mode):
    if dt_mode in _CACHE:
        return _CACHE[dt_mode]

    import concourse.bass as bass
    import concourse.tile as tile
    from concourse import bacc, mybir

    f32 = mybir.dt.float32
    DT = mybir.dt.float32r if dt_mode == "f32r" else f32
    Alu = mybir.AluOpType

    nc = bacc.Bacc("TRN2", target_bir_lowering=False, debug=False,
                   num_devices=N_CORES)

    x_d = nc.dram_tensor("x", (LANES, T), f32, kind="ExternalInput").ap()
    t1_d = nc.dram_tensor("toep1", (128, 3 * 128), f32, kind="ExternalInput").ap()
    t2_d = nc.dram_tensor("toep2", (128, 4 * MM2_N), f32, kind="ExternalInput").ap()
    id_d = nc.dram_tensor("ident", (128, 128), f32, kind="ExternalInput").ap()
    tm_d = nc.dram_tensor("tailmask", (128, 1), f32, kind="ExternalInput").ap()
    y_d = nc.dram_tensor("y", (LANES, T), f32, kind="ExternalOutput").ap()

    with tile.TileContext(nc) as tc:
        with (
            tc.tile_pool(name="const", bufs=1) as constp,
            tc.tile_pool(name="xs", bufs=3) as xsp,
            tc.tile_pool(name="stage", bufs=3) as stagep,
            tc.tile_pool(name="persist", bufs=1) as persist,
            tc.tile_pool(name="small", bufs=4) as smallp,
            tc.tile_pool(name="ptp", bufs=2, space="PSUM") as ptp,
            tc.tile_pool(name="pm1", bufs=2, space="PSUM") as pm1,
            tc.tile_pool(name="pm2", bufs=2, space="PSUM") as pm2,
        ):
            # ---- constants ----
            ident = constp.tile([128, 128], f32)
            nc.sync.dma_start(ident[:], id_d[:])
            tmask = constp.tile([128, 1], f32)
            nc.sync.dma_start(tmask[:], tm_d[:])
            t1f = constp.tile([128, 3, 128], f32)
            nc.sync.dma_start(t1f[:], t1_d.rearrange("k (c m) -> k c m", c=3))
            t2f = constp.tile([128, 4, MM2_N], f32)
            nc.sync.dma_start(t2f[:], t2_d.rearrange("k (c j) -> k c j", c=4))
            if DT is not f32:
                t1 = constp.tile([128, 3, 128], DT)
                nc.vector.tensor_copy(t1[:], t1f[:])
                t2 = constp.tile([128, 4, MM2_N], DT)
                nc.vector.tensor_copy(t2[:], t2f[:])
            else:
                t1, t2 = t1f, t2f

            # ---- pass A: per-lane max|x| ----
            smax = persist.tile([128, NSTRIPS], f32)
            for i in range(NSTRIPS):
                lo = i * STRIP
                hi = min(T, lo + STRIP)
                if lo >= T:
                    nc.vector.memset(smax[:, i:i + 1], 0.0)
                    continue
                xa = xsp.tile([128, STRIP], f32, tag="xstrip")
                nc.sync.dma_start(xa[:, 0:hi - lo], x_d[:, lo:hi])
                nc.vector.reduce_max(smax[:, i:i + 1], xa[:, 0:hi - lo],
                                     axis=mybir.AxisListType.X,
                                     apply_absolute_value=True)
            s_pos = persist.tile([128, 1], f32)
            nc.vector.reduce_max(s_pos[:], smax[:], axis=mybir.AxisListType.X)
            s_neg = persist.tile([128, 1], f32)
            nc.scalar.mul(s_neg[:], s_pos[:], -1.0)

            # ---- persistent stream buffers ----
            st_buf = persist.tile([128, UNITS + 2, 128], DT)   # time-major x
            yt_a = persist.tile([128, UNITS, 128], DT, tag="yt_a")
            yt_b = persist.tile([128, UNITS, 128], DT, tag="yt_b")
            yt_bufs = [yt_a, yt_b]
            nc.vector.memset(st_buf[:, 0:2, :], 0.0)  # tiles -2,-1 of stream

            def emit_mm2(i, j):
                """backward conv for stream tiles (16i+2j, +1) -> clamp -> stage."""
                tau0 = 16 * i + 2 * j
                p2 = pm2.tile([128, MM2_N], f32, tag="p2")
                for c in range(4):
                    sl = 2 * j + c
                    if sl < UNITS:
                        lhs = yt_bufs[i % 2][:, sl, :]
                    else:
                        lhs = yt_bufs[(i + 1) % 2][:, sl - UNITS, :]
                    nc.tensor.matmul(p2[:], lhs, t2[:, c, :],
                                     start=(c == 0), stop=(c == 3))
                stg = stages[i]
                nc.vector.tensor_scalar(
                    stg[:, 2 * j * 128:(2 * j + 2) * 128], p2[:],
                    s_pos[:], s_neg[:], Alu.min, Alu.max)

            def flush_stage(i):
                stg = stages[i]
                lo = i * STRIP - PADLEN
                hi = min(T, lo + STRIP)
                olo = max(0, lo)
                nc.sync.dma_start(y_d[:, olo:hi], stg[:, olo - lo:hi - lo])

            stages = {}

            # ---- pass B ----
            for i in range(NSTRIPS):
                s0 = i * STRIP
                xb = xsp.tile([128, STRIP], f32, tag="xstrip")
                # load raw x into stream positions [s0, s0+STRIP) (offset -9)
                if i == 0:
                    nc.sync.dma_start(xb[:, PADLEN:STRIP],
                                      x_d[:, 0:STRIP - PADLEN])
                    two_x0 = smallp.tile([128, 1], f32, tag="twox")
                    nc.scalar.mul(two_x0[:], xb[:, PADLEN:PADLEN + 1], 2.0)
                    nc.vector.tensor_scalar(
                        xb[:, 0:PADLEN],
                        xb[:, 2 * PADLEN - 1:PADLEN - 1:-1],
                        -1.0, two_x0[:], Alu.mult, Alu.add)
                elif i < NSTRIPS - 1:
                    nc.sync.dma_start(xb[:], x_d[:, s0 - PADLEN:s0 + STRIP - PADLEN])
                else:
                    nval = T - (s0 - PADLEN)     # 905
                    nc.sync.dma_start(xb[:, 0:nval], x_d[:, s0 - PADLEN:T])
                    two_xe = smallp.tile([128, 1], f32, tag="twox")
                    nc.scalar.mul(two_xe[:], xb[:, nval - 1:nval], 2.0)
                    nc.vector.tensor_scalar(
                        xb[:, nval:nval + PADLEN],
                        xb[:, nval - 3:nval - 12:-1],
                        -1.0, two_xe[:], Alu.mult, Alu.add)
                    nc.vector.memset(xb[:, nval + PADLEN:STRIP], 0.0)

                n_units = UNITS if i < NSTRIPS - 1 else 8
                n_g1 = 4 if i < NSTRIPS - 1 else 2

                # transpose to time-major, 4 tiles per PSUM bank
                for v0 in range(0, n_units, 4):
                    tp = ptp.tile([128, 4, 128], f32, tag="tp")
                    for v in range(4):
                        if v0 + v < n_units:
                            nc.tensor.transpose(
                                tp[:, v, :], xb[:, (v0 + v) * 128:(v0 + v + 1) * 128],
                                ident[:])
                    nc.scalar.copy(st_buf[:, 2 + v0:2 + v0 + 4, :], tp[:])

                # MM1: forward conv, groups of 4 output tiles
                ycur = yt_bufs[i % 2]
                for g in range(n_g1):
                    p1 = pm1.tile([128, 4, 128], f32, tag="p1")
                    for c in range(3):
                        nc.tensor.matmul(
                            p1[:], t1[:, c, :],
                            st_buf[:, 4 * g + c:4 * g + c + 4, :],
                            start=(c == 0), stop=(c == 2))
                    if i == NSTRIPS - 1 and g == n_g1 - 1:
                        # forward stream must be exactly 0 beyond TP=48018:
                        # tile 375 keeps only its first 18 time positions
                        nc.scalar.copy(ycur[:, 4 * g:4 * g + 3, :], p1[:, 0:3, :])
                        nc.vector.tensor_scalar(
                            ycur[:, 4 * g + 3, :], p1[:, 3, :],
                            tmask[:], None, Alu.mult)
                    else:
                        nc.scalar.copy(ycur[:, 4 * g:4 * g + 4, :], p1[:])

                if i == NSTRIPS - 1:
                    nc.vector.memset(ycur[:, 8:UNITS, :], 0.0)

                # carry last two time-major tiles to slots 0,1 for next strip
                if i < NSTRIPS - 1:
                    nc.vector.tensor_copy(st_buf[:, 0:2, :],
                                          st_buf[:, UNITS:UNITS + 2, :])

                # MM2 for all groups whose forward inputs now exist
                stages[i] = stagep.tile([128, STRIP], f32, tag="stage", name=f"stage{i}")
                if i > 0:
                    emit_mm2(i - 1, 7)
                    flush_stage(i - 1)
                last_j = 7 if i < NSTRIPS - 1 else 4
                for j in range(0, last_j):
                    emit_mm2(i, j)
            flush_stage(NSTRIPS - 1)

    nc.compile()
    _CACHE[dt_mode] = nc
    return nc


def kernel(x, b, a):
    x = np.ascontiguousarray(np.asarray(x, dtype=np.float32))
    shape = x.shape
    xl = x.reshape(LANES_TOTAL, T)

    toep1, toep2 = _tables(np.asarray(b), np.asarray(a))
    ident = np.eye(128, dtype=np.float32)
    tailmask = np.zeros((128, 1), dtype=np.float32)
    tailmask[0:TP - 128 * (NT_VALID - 1)] = 1.0  # first 18 rows

    nc = _build(DT_MODE)

    from concourse import bass_utils
    in_maps = []
    for c in range(N_CORES):
        in_maps.append({
            "x": np.ascontiguousarray(xl[c * LANES:(c + 1) * LANES]),
            "toep1": toep1, "toep2": toep2, "ident": ident,
            "tailmask": tailmask,
        })
    res = bass_utils.run_bass_kernel_spmd(nc, in_maps,
                                          core_ids=list(range(N_CORES)))
    out = np.concatenate([r["y"] for r in res.results], axis=0)
    return out.reshape(shape)
